# revision 4
# baseline (speedup 1.0000x reference)
"""Trainium2 Bass kernel for nn_BasicNet4 (Emformer encoder, sparse attention).

Strategy (v3):
  - Data-parallel over batch B=8 across 8 NeuronCores (weights replicated).
  - Tokens reordered host-side into segment-interleaved order:
    seg i -> [rc_i, u_{4i}, u_{4i+1}, u_{4i+2}, u_{4i+3}]  (5 tokens x 256 segs)
    so attention is block-diagonal with 5x5 blocks.
  - Attention in 125-query / 128-key windows: identical block-diagonal mask in
    every window, no edge/halo handling.
  - Activations transposed in SBUF: [d on partitions (4x128), tokens on free].
    LN stats via ones-matmul partition reductions (broadcast form).
  - s-stream reparameterization (scalar ln_out affine folded into next-layer
    weights + final host affine): ln_in of layers 1..3 is free.
  - ff_ln folded into the FFN: W1 runs directly on the *uncentered* residual;
    the exact rank-1 correction (row-sums of W1 x mean) and the per-token rstd
    are applied at PSUM-drain time on the DVE.  W1 never waits for LN stats.
  - V bias folded into Wo bias (attention rows sum to 1).
  - Residual adds + biases fused into single DVE scalar_tensor_tensor drains.
  - Softmax reciprocal via reciprocal_approx_fast; LN chains kept in bf16
    SBUF operands for fast DVE modes.
"""

import sys

sys.path.insert(0, "/opt/trn_rl_repo")

import numpy as np
import ml_dtypes

import concourse.bass as bass
import concourse.mybir as mybir
import concourse.tile as tile
from concourse import bass_utils, bacc

bf16 = ml_dtypes.bfloat16
dt = mybir.dt
AF = mybir.ActivationFunctionType
ALU = mybir.AluOpType

# Model config (hardcoded from the problem spec)
D, H, FFN, L = 512, 4, 128, 4
SEG, RC = 4, 1
B, T = 8, 1025
U = T - RC            # 1024
NSEG = U // SEG       # 256
TT = NSEG * (SEG + RC)  # 1280 interleaved tokens
DT = D // 128         # 4 d tiles
DH = D // H           # 128 (= one partition tile per head)
NCORES = 8
CHUNKS = [(0, 512), (512, 512), (1024, 256)]  # free-dim chunks <= 512 (PSUM bank)

WQ = 125              # query-window stride (25 whole segments)
KW = 128              # key-window width
NW = -(-TT // WQ)     # 11 windows (last one is 30 tokens)
RANK = 1 + (KW // 5)  # 26: mask factorization rank (1 bias row + 25 segs)
GROUPS = [list(range(4 * g, min(4 * g + 4, NW))) for g in range(-(-NW // 4))]

CBF = np.float32(bf16(np.float32(1e9)))  # mask constant, exact in bf16

_COMPILED = None
_FAST = None


def _tok_index():
    # interleaved token t -> original frame index in x[:, :T]
    t = np.arange(TT)
    seg = t // 5
    pos = t % 5
    off = np.array([4, 0, 1, 2, 3])[pos]
    return 4 * seg + off  # in [0, 1024]


def _win_geom(w):
    q0 = WQ * w
    qn = min(KW, TT - q0)   # query stream width (masked beyond 125)
    kn = min(KW, TT - q0)   # key window width
    return q0, qn, kn


def _mask_consts():
    """lmask [RANK,128] (lhsT), rmask [RANK,128*NW] (rhs):
    sum_r lmask[r,m]*rmask[r, 128w+j] = -C + C*[m//5 == j//5] for real in-window
    query cols j<125 (within bounds), -C for pad/overhang cols."""
    lm = np.zeros((RANK, KW), np.float32)
    lm[0, :] = 1.0
    segk = np.arange(KW) // 5          # 0..25 (seg 25 has no indicator row)
    for i in range(25):
        lm[1 + i, :] = (segk == i)
    rm = np.zeros((RANK, 128 * NW), np.float32)
    for w in range(NW):
        q0, qn, _ = _win_geom(w)
        nreal = min(WQ, TT - q0)       # real query cols in this window
        col = 128 * w
        rm[0, col:col + 128] = -CBF
        for j in range(nreal):
            rm[1 + (j // 5), col + j] = CBF
    return lm.astype(bf16), rm.astype(bf16)


def _fast_ok(ins):
    """Fast path: ln_out gain/bias scalar (and gain>0) for layers 0..L-2."""
    f32 = np.float32
    for l in range(L - 1):
        g = f32(ins["ln_out_g"][l])
        b = f32(ins["ln_out_b"][l])
        if not (np.all(g == g[0]) and g[0] > 0 and np.all(b == b[0])):
            return False
    return True


def _host_prep(ins, fast):
    """Fold LN affines/scales into weights, transpose, cast. Shared input map."""
    f32 = np.float32
    m = {}
    scale = np.float32(DH) ** -0.5
    for l in range(L):
        g_i, b_i = f32(ins["ln_in_g"][l]), f32(ins["ln_in_b"][l])
        g_f, b_f = f32(ins["ff_ln_g"][l]), f32(ins["ff_ln_b"][l])
        Wq = f32(ins["Wq"][l]);  bq = f32(ins["bq"][l])
        Wk = f32(ins["Wkv"][l][:D]);  bk = f32(ins["bkv"][l][:D])
        Wv = f32(ins["Wkv"][l][D:]);  bv = f32(ins["bkv"][l][D:])
        Wo = f32(ins["Wo"][l]);  bo = f32(ins["bo"][l])
        W1 = f32(ins["W1"][l]);  b1 = f32(ins["b1"][l])
        W2 = f32(ins["W2"][l]);  b2 = f32(ins["b2"][l])
        gp = f32(1.0)
        if fast and l > 0:
            gp = f32(ins["ln_out_g"][l - 1][0])   # scalar, >0 (checked)
        Wq_ = scale * (Wq * g_i[None, :]); bq_ = scale * (bq + Wq @ b_i)
        Wk_ = Wk * g_i[None, :];           bk_ = bk + Wk @ b_i
        Wv_ = Wv * g_i[None, :];           bv_ = bv + Wv @ b_i
        Wo_ = Wo / gp;                     bo_ = (bo + Wo @ bv_) / gp
        W1_ = W1 * g_f[None, :];           b1_ = b1 + W1 @ b_f
        W2_ = W2 / gp;                     b2_ = b2 / gp
        m[f"wq{l}"] = Wq_.T.copy().astype(bf16)   # [din, dout]
        m[f"wk{l}"] = Wk_.T.copy().astype(bf16)
        m[f"wv{l}"] = Wv_.T.copy().astype(bf16)
        m[f"wo{l}"] = Wo_.T.copy().astype(bf16)
        m[f"w1{l}"] = W1_.T.copy().astype(bf16)   # [512, 128]
        m[f"w2{l}"] = W2_.T.copy().astype(bf16)   # [128, 512]
        m[f"bq{l}"] = bq_.reshape(DT, 128).T.copy()       # [128, DT] f32
        m[f"bk{l}"] = bk_.reshape(DT, 128).T.copy()
        m[f"bo{l}"] = bo_.reshape(DT, 128).T.copy()
        m[f"b1{l}"] = b1_.reshape(1, FFN).T.copy()        # [128, 1]
        m[f"b2{l}"] = b2_.reshape(DT, 128).T.copy()
        m[f"w1r{l}"] = (-W1_.sum(axis=1)).reshape(1, FFN).T.copy()  # [128,1] -rowsum
        if not fast:
            m[f"go{l}"] = f32(ins["ln_out_g"][l]).reshape(DT, 128).T.copy()
            m[f"bo2{l}"] = f32(ins["ln_out_b"][l]).reshape(DT, 128).T.copy()
    lm, rm = _mask_consts()
    m["lmask"] = lm
    m["rmask"] = rm
    m["ones_c"] = np.full((128, 128), 1.0 / D, bf16)  # stats lhsT (bcast mean)
    m["allones"] = np.ones((128, 128), bf16)          # softmax denominator lhsT
    return m


def _dram_inputs(nc, fast):
    a = {}
    def inp(name, shape, dtype):
        a[name] = nc.dram_tensor(name, list(shape), dtype, kind="ExternalInput").ap()
    inp("xT", (D, TT), dt.bfloat16)
    for l in range(L):
        inp(f"wq{l}", (D, D), dt.bfloat16); inp(f"wk{l}", (D, D), dt.bfloat16)
        inp(f"wv{l}", (D, D), dt.bfloat16); inp(f"wo{l}", (D, D), dt.bfloat16)
        inp(f"w1{l}", (D, FFN), dt.bfloat16); inp(f"w2{l}", (FFN, D), dt.bfloat16)
        inp(f"bq{l}", (128, DT), dt.float32); inp(f"bk{l}", (128, DT), dt.float32)
        inp(f"bo{l}", (128, DT), dt.float32)
        inp(f"b1{l}", (128, 1), dt.float32); inp(f"b2{l}", (128, DT), dt.float32)
        inp(f"w1r{l}", (128, 1), dt.float32)
        if not fast:
            inp(f"go{l}", (128, DT), dt.float32)
            inp(f"bo2{l}", (128, DT), dt.float32)
    inp("lmask", (RANK, KW), dt.bfloat16)
    inp("rmask", (RANK, 128 * NW), dt.bfloat16)
    inp("ones_c", (128, 128), dt.bfloat16)
    inp("allones", (128, 128), dt.bfloat16)
    out = nc.dram_tensor("out", [128, DT], dt.float32, kind="ExternalOutput").ap()
    return a, out


def _trace(nc, fast):
    a, out_dram = _dram_inputs(nc, fast)
    with tile.TileContext(nc) as tc:
        import contextlib
        ctx = contextlib.ExitStack()
        with ctx:
            consts = ctx.enter_context(tc.tile_pool(name="consts", bufs=1))
            wpool = ctx.enter_context(tc.tile_pool(name="w", bufs=2))
            acts = ctx.enter_context(tc.tile_pool(name="acts", bufs=1))
            small = ctx.enter_context(tc.tile_pool(name="small", bufs=2))
            psum = ctx.enter_context(tc.tile_pool(name="psum", bufs=1, space="PSUM"))

            # ---- constants ----
            smalls = {}
            for name, shape, dd in [
                ("lmask", [RANK, KW], dt.bfloat16),
                ("rmask", [RANK, 128 * NW], dt.bfloat16),
                ("ones_c", [128, 128], dt.bfloat16),
                ("allones", [128, 128], dt.bfloat16),
            ]:
                t = consts.tile(shape, dd, tag=name, name=name)
                nc.sync.dma_start(t[:], a[name])
                smalls[name] = t
            eps_tile = consts.tile([128, 1], dt.float32)
            nc.vector.memset(eps_tile[:], 1e-5)
            ones_c, allones = smalls["ones_c"], smalls["allones"]
            lmask, rmask = smalls["lmask"], smalls["rmask"]

            def emit_sq(src, sq, c0, cn):
                """sq = src*src for one chunk, split across DVE/GPSIMD."""
                for d in range(DT):
                    eng = nc.gpsimd if d == DT - 1 else nc.vector
                    eng.tensor_tensor(sq[:, d, c0:c0 + cn], src[:, d, c0:c0 + cn],
                                      src[:, d, c0:c0 + cn], ALU.mult)

            def ln_stats_chunk(src, sq, c0, cn, need_A_bf):
                """Per-chunk LN stats: returns (pmu, mu_sb, A_f32slot, A_bf).
                pmu stays valid until its ring slot is reused."""
                pmu = psum.tile([128, 512], dt.float32, tag="pp", bufs=4, name="pmu")
                pe2 = psum.tile([128, 512], dt.float32, tag="pp", bufs=4, name="pe2")
                for d in range(DT):
                    nc.tensor.matmul(pmu[:, :cn], ones_c[:], src[:, d, c0:c0 + cn],
                                     start=(d == 0), stop=(d == DT - 1))
                for d in range(DT):
                    nc.tensor.matmul(pe2[:, :cn], ones_c[:], sq[:, d, c0:c0 + cn],
                                     start=(d == 0), stop=(d == DT - 1))
                mu_sb = small.tile([128, 512], dt.bfloat16, tag="musb", name="musb")
                nc.vector.tensor_copy(mu_sb[:, :cn], pmu[:, :cn])
                sqmu = small.tile([128, 512], dt.float32, tag="sqmu", name="sqmu")
                nc.vector.tensor_tensor(sqmu[:, :cn], mu_sb[:, :cn], mu_sb[:, :cn],
                                        ALU.mult)
                var = small.tile([128, 512], dt.float32, tag="var", name="var")
                nc.vector.scalar_tensor_tensor(var[:, :cn], sqmu[:, :cn], -1.0,
                                               pe2[:, :cn], ALU.mult, ALU.add)
                sd = small.tile([128, 512], dt.float32, tag="sd", name="sd")
                nc.scalar.activation(sd[:, :cn], var[:, :cn], AF.Sqrt,
                                     bias=eps_tile[:], scale=1.0)
                A = small.tile([128, 512], dt.float32, tag="A", bufs=3, name="A")
                nc.vector.reciprocal_approx_fast(A[:, :cn], sd[:, :cn])
                A_bf = None
                if need_A_bf:
                    A_bf = small.tile([128, 512], dt.bfloat16, tag="Abf", bufs=3,
                                      name="Abf")
                    nc.vector.tensor_copy(A_bf[:, :cn], A[:, :cn])
                return pmu, mu_sb, A, A_bf

            def ln_apply(src, dst):
                """dst = (src - mean) * rstd (per token)."""
                sq = acts.tile([128, DT, TT], dt.bfloat16, tag="sq", name="sq")
                for (c0, cn) in CHUNKS:
                    emit_sq(src, sq, c0, cn)
                    pmu, mu_sb, A, A_bf = ln_stats_chunk(src, sq, c0, cn, True)
                    for d in range(DT):
                        t2 = small.tile([128, 512], dt.bfloat16, tag="t2", name="t2")
                        nc.vector.scalar_tensor_tensor(
                            t2[:, :cn], mu_sb[:, :cn], -1.0, src[:, d, c0:c0 + cn],
                            ALU.mult, ALU.add)
                        nc.vector.tensor_tensor(dst[:, d, c0:c0 + cn], t2[:, :cn],
                                                A_bf[:, :cn], ALU.mult)

            # ---- initial residual: raw x (interleaved, transposed), chunked ----
            res = acts.tile([128, DT, TT], dt.bfloat16, tag="res", bufs=2, name="res")
            xTr = a["xT"].rearrange("(dtile p) t -> p dtile t", p=128)
            for (c0, cn) in CHUNKS:
                nc.sync.dma_start(res[:, :, c0:c0 + cn], xTr[:, :, c0:c0 + cn])

            for l in range(L):
                # ---- layer weights ----
                w = {}
                for nm, shape in [("wq", [128, DT, D]), ("wk", [128, DT, D]),
                                  ("wv", [128, DT, D]), ("wo", [128, DT, D]),
                                  ("w1", [128, DT, FFN])]:
                    t = wpool.tile(shape, dt.bfloat16, tag=nm, name=nm)
                    nc.sync.dma_start(t[:], a[f"{nm}{l}"].rearrange(
                        "(dtile p) o -> p dtile o", p=128))
                    w[nm] = t
                w["w2"] = wpool.tile([128, D], dt.bfloat16, tag="w2", name="w2")
                nc.sync.dma_start(w["w2"][:], a[f"w2{l}"])
                bias = {}
                bnames = ["bq", "bk", "bo", "b1", "b2", "w1r"] + (
                    [] if fast else ["go", "bo2"])
                for nm in bnames:
                    t = wpool.tile([128, DT] if nm not in ("b1", "w1r") else [128, 1],
                                   dt.float32, tag=nm, name=nm)
                    nc.sync.dma_start(t[:], a[f"{nm}{l}"])
                    bias[nm] = t

                # ---- ln_in (explicit for layer 0 / general path) ----
                if l == 0 or not fast:
                    zq = acts.tile([128, DT, TT], dt.bfloat16, tag="zq", name="zq")
                    ln_apply(res, zq)
                else:
                    zq = res

                # ---- Q, K projections (weights stationary, transposed out) ----
                qk = {}
                for nm, bnm, tg in [("wq", "bq", "qt"), ("wk", "bk", "kt")]:
                    dst = acts.tile([128, DT, TT], dt.bfloat16, tag=tg, name=tg)
                    for o in range(DT):
                        for (c0, cn) in CHUNKS:
                            p = psum.tile([128, 512], dt.float32, tag="pp", bufs=4,
                                          name="pqk")
                            for d in range(DT):
                                nc.tensor.matmul(
                                    p[:, :cn],
                                    w[nm][:, d, 128 * o:128 * o + 128],
                                    zq[:, d, c0:c0 + cn],
                                    start=(d == 0), stop=(d == DT - 1))
                            nc.scalar.activation(dst[:, o, c0:c0 + cn], p[:, :cn],
                                                 AF.Identity,
                                                 bias=bias[bnm][:, o:o + 1], scale=1.0)
                    qk[nm] = dst
                q_t, k_t = qk["wq"], qk["wk"]

                # ---- V in overlapping 128-token key windows (no bias: folded) ----
                v_win = acts.tile([128, NW, D], dt.bfloat16, tag="vw", name="vw")
                for wi in range(NW):
                    kw0, _, kn = _win_geom(wi)
                    p = psum.tile([128, 512], dt.float32, tag="pp", bufs=4, name="pv")
                    for d in range(DT):
                        nc.tensor.matmul(p[0:kn, :], zq[:, d, kw0:kw0 + kn],
                                         w["wv"][:, d, :],
                                         start=(d == 0), stop=(d == DT - 1))
                    nc.scalar.activation(v_win[0:kn, wi, :], p[0:kn, :], AF.Identity)

                # ---- attention: per head, per 4-window group ----
                attn = acts.tile([128, DT, TT], dt.bfloat16, tag="at", name="at")
                for h in range(H):
                    for g, wlist in enumerate(GROUPS):
                        ng = 128 * len(wlist)
                        ps = psum.tile([128, 512], dt.float32, tag="ps", bufs=2,
                                       name="ps")
                        nc.tensor.matmul(ps[:, :ng], lmask[:],
                                         rmask[:, 512 * g:512 * g + ng],
                                         start=True, stop=False)
                        for wi in wlist:
                            q0, qn, kn = _win_geom(wi)
                            ow = 128 * (wi - wlist[0])
                            nc.tensor.matmul(ps[0:kn, ow:ow + qn],
                                             k_t[:, h, q0:q0 + kn],
                                             q_t[:, h, q0:q0 + qn],
                                             start=False, stop=True)
                        pa = small.tile([128, 512], dt.bfloat16, tag="pa", bufs=3,
                                        name="pa")
                        nc.scalar.activation(pa[:, :ng], ps[:, :ng], AF.Exp)
                        pd = psum.tile([128, 512], dt.float32, tag="pd", bufs=1,
                                       name="pd")
                        nc.tensor.matmul(pd[:, :ng], allones[:], pa[:, :ng],
                                         start=True, stop=True)
                        rec = small.tile([128, 512], dt.float32, tag="rec", name="rec")
                        nc.vector.reciprocal_approx_fast(rec[:, :ng], pd[:, :ng])
                        pav = psum.tile([128, 512], dt.float32, tag="pav", bufs=1,
                                        name="pav")
                        for wi in wlist:
                            q0, qn, kn = _win_geom(wi)
                            ow = 128 * (wi - wlist[0])
                            nc.tensor.matmul(pav[:, ow:ow + qn],
                                             v_win[0:kn, wi, 128 * h:128 * h + 128],
                                             pa[0:kn, ow:ow + qn],
                                             start=True, stop=True)
                        # normalize + compact (drop per-window pad/overhang cols)
                        full = [wi for wi in wlist if WQ * wi + WQ <= TT]
                        nf = len(full)
                        qg0 = WQ * wlist[0]
                        if nf:
                            pav_v = pav[:, :].rearrange("p (w j) -> p w j", j=128)
                            rec_v = rec[:, :].rearrange("p (w j) -> p w j", j=128)
                            nc.vector.tensor_tensor(
                                attn[:, h, qg0:qg0 + WQ * nf],
                                pav_v[:, 0:nf, 0:WQ], rec_v[:, 0:nf, 0:WQ], ALU.mult)
                        for wi in wlist[nf:]:          # partial tail window
                            q0, qn, _ = _win_geom(wi)
                            ow = 128 * (wi - wlist[0])
                            nc.vector.tensor_tensor(
                                attn[:, h, q0:TT],
                                pav[:, ow:ow + (TT - q0)],
                                rec[:, ow:ow + (TT - q0)], ALU.mult)

                # ---- Wo projection + bias + residual (fused drain) ----
                rc = acts.tile([128, DT, TT], dt.bfloat16, tag="rc", name="rc")
                for o in range(DT):
                    for (c0, cn) in CHUNKS:
                        p = psum.tile([128, 512], dt.float32, tag="pp", bufs=4,
                                      name="pwo")
                        for d in range(DT):
                            nc.tensor.matmul(p[:, :cn],
                                             w["wo"][:, d, 128 * o:128 * o + 128],
                                             attn[:, d, c0:c0 + cn],
                                             start=(d == 0), stop=(d == DT - 1))
                        nc.vector.scalar_tensor_tensor(
                            rc[:, o, c0:c0 + cn], p[:, :cn], bias["bo"][:, o:o + 1],
                            res[:, o, c0:c0 + cn], ALU.add, ALU.add)

                # ---- FFN with ff_ln folded into the drains ----
                # h1 = relu((W1@rc - w1_rowsum*mu)*A + b1); W1 needs no LN stats.
                sqf = acts.tile([128, DT, TT], dt.bfloat16, tag="sq", name="sqf")
                h1 = acts.tile([128, TT], dt.bfloat16, tag="h1", name="h1")
                for (c0, cn) in CHUNKS:
                    p1 = psum.tile([128, 512], dt.float32, tag="pp", bufs=4,
                                   name="p1")
                    for d in range(DT):
                        nc.tensor.matmul(p1[:, :cn], w["w1"][:, d, :],
                                         rc[:, d, c0:c0 + cn],
                                         start=(d == 0), stop=(d == DT - 1))
                    emit_sq(rc, sqf, c0, cn)
                    pmu, mu_sb, A, _ = ln_stats_chunk(rc, sqf, c0, cn, False)
                    muA = small.tile([128, 512], dt.bfloat16, tag="muA", name="muA")
                    nc.vector.tensor_tensor(muA[:, :cn], pmu[:, :cn], A[:, :cn],
                                            ALU.mult)
                    t1 = small.tile([128, 512], dt.bfloat16, tag="t1", name="t1")
                    nc.vector.tensor_tensor(t1[:, :cn], p1[:, :cn], A[:, :cn],
                                            ALU.mult)
                    hp = small.tile([128, 512], dt.bfloat16, tag="hp", name="hp")
                    nc.vector.scalar_tensor_tensor(hp[:, :cn], muA[:, :cn],
                                                   bias["w1r"][:], t1[:, :cn],
                                                   ALU.mult, ALU.add)
                    nc.vector.tensor_scalar(h1[:, c0:c0 + cn], hp[:, :cn],
                                            bias["b1"][:], 0.0, ALU.add, ALU.max)
                y = acts.tile([128, DT, TT], dt.bfloat16, tag="y", name="y")
                for o in range(DT):
                    for (c0, cn) in CHUNKS:
                        p = psum.tile([128, 512], dt.float32, tag="pp", bufs=4,
                                      name="p2")
                        nc.tensor.matmul(p[:, :cn], w["w2"][:, 128 * o:128 * o + 128],
                                         h1[:, c0:c0 + cn], start=True, stop=True)
                        nc.vector.scalar_tensor_tensor(
                            y[:, o, c0:c0 + cn], p[:, :cn], bias["b2"][:, o:o + 1],
                            rc[:, o, c0:c0 + cn], ALU.add, ALU.add)

                # ---- ln_out -> next residual (normalized s-stream) ----
                s_next = acts.tile([128, DT, TT], dt.bfloat16, tag="res", bufs=2,
                                   name="snext")
                ln_apply(y, s_next)
                if not fast:
                    for d in range(DT):
                        nc.vector.tensor_scalar(s_next[:, d], s_next[:, d],
                                                bias["go"][:, d:d + 1],
                                                bias["bo2"][:, d:d + 1],
                                                ALU.mult, ALU.add)
                res = s_next

            # ---- mean-pool utterance tokens (pos 1..4 of each 5-block) ----
            out_sb = small.tile([128, DT], dt.float32, tag="outsb", name="outsb")
            for d in range(DT):
                view = res[:, d, :].rearrange("p (s j) -> p s j", j=5)[:, :, 1:5]
                nc.vector.tensor_reduce(out_sb[:, d:d + 1], view,
                                        axis=mybir.AxisListType.XY, op=ALU.add)
            nc.vector.tensor_scalar_mul(out_sb[:], out_sb[:], 1.0 / U)
            nc.sync.dma_start(out_dram, out_sb[:])
    nc.compile()
    return nc


def _build(fast):
    nc = bacc.Bacc("TRN2", target_bir_lowering=False, debug=False, num_devices=NCORES)
    return _trace(nc, fast)


def kernel(**inputs):
    global _COMPILED, _FAST
    ins = {k: np.asarray(v) for k, v in inputs.items()}
    fast = _fast_ok(ins)
    shared = _host_prep(ins, fast)
    idx = _tok_index()
    x = ins["x"].astype(np.float32)          # [B, T, D]
    xp = x[:, idx, :]                        # [B, TT, D]
    xT = np.ascontiguousarray(xp.transpose(0, 2, 1)).astype(bf16)  # [B, D, TT]
    if _COMPILED is None or _FAST != fast:
        _COMPILED = _build(fast)
        _FAST = fast
    nc = _COMPILED
    in_maps = []
    for b in range(NCORES):
        m = dict(shared)
        m["xT"] = xT[b]
        in_maps.append(m)
    res = bass_utils.run_bass_kernel_spmd(nc, in_maps, core_ids=list(range(NCORES)))
    outs = []
    for b in range(NCORES):
        o = res.results[b]["out"]            # [128, DT]
        outs.append(o.T.reshape(D))          # d = dtile*128 + p
    out = np.stack(outs).astype(np.float32)
    if fast:
        g3 = np.float32(ins["ln_out_g"][L - 1])
        b3 = np.float32(ins["ln_out_b"][L - 1])
        out = out * g3[None, :] + b3[None, :]
    return out


# revision 14
# speedup vs baseline: 1.0016x; 1.0016x over previous
"""Trainium2 Bass kernel for nn_BasicNet4 (Emformer encoder, sparse attention).

Strategy (v3):
  - Data-parallel over batch B=8 across 8 NeuronCores (weights replicated).
  - Tokens reordered host-side into segment-interleaved order:
    seg i -> [rc_i, u_{4i}, u_{4i+1}, u_{4i+2}, u_{4i+3}]  (5 tokens x 256 segs)
    so attention is block-diagonal with 5x5 blocks.
  - Attention in 125-query / 128-key windows: identical block-diagonal mask in
    every window, no edge/halo handling.
  - Activations transposed in SBUF: [d on partitions (4x128), tokens on free].
    LN stats via ones-matmul partition reductions (broadcast form).
  - s-stream reparameterization (scalar ln_out affine folded into next-layer
    weights + final host affine): ln_in of layers 1..3 is free.
  - ff_ln folded into the FFN: W1 runs directly on the *uncentered* residual;
    the exact rank-1 correction (row-sums of W1 x mean) and the per-token rstd
    are applied at PSUM-drain time on the DVE.  W1 never waits for LN stats.
  - V bias folded into Wo bias (attention rows sum to 1).
  - Residual adds + biases fused into single DVE scalar_tensor_tensor drains.
  - Softmax reciprocal via reciprocal_approx_fast; LN chains kept in bf16
    SBUF operands for fast DVE modes.
"""

import sys

sys.path.insert(0, "/opt/trn_rl_repo")

import numpy as np
import ml_dtypes

import concourse.bass as bass
import concourse.mybir as mybir
import concourse.tile as tile
from concourse import bass_utils, bacc

bf16 = ml_dtypes.bfloat16
dt = mybir.dt
AF = mybir.ActivationFunctionType
ALU = mybir.AluOpType

# Model config (hardcoded from the problem spec)
D, H, FFN, L = 512, 4, 128, 4
SEG, RC = 4, 1
B, T = 8, 1025
U = T - RC            # 1024
NSEG = U // SEG       # 256
TT = NSEG * (SEG + RC)  # 1280 interleaved tokens
DT = D // 128         # 4 d tiles
DH = D // H           # 128 (= one partition tile per head)
NCORES = 8
CHUNKS = [(0, 512), (512, 512), (1024, 256)]  # free-dim chunks <= 512 (PSUM bank)

WQ = 125              # query-window stride (25 whole segments)
KW = 128              # key-window width
NW = -(-TT // WQ)     # 11 windows (last one is 30 tokens)
RANK = 1 + (KW // 5)  # 26: mask factorization rank (1 bias row + 25 segs)
GROUPS = [list(range(4 * g, min(4 * g + 4, NW))) for g in range(-(-NW // 4))]

CBF = np.float32(bf16(np.float32(1e9)))  # mask constant, exact in bf16

_COMPILED = None
_FAST = None


def _tok_index():
    # interleaved token t -> original frame index in x[:, :T]
    t = np.arange(TT)
    seg = t // 5
    pos = t % 5
    off = np.array([4, 0, 1, 2, 3])[pos]
    return 4 * seg + off  # in [0, 1024]


def _win_geom(w):
    q0 = WQ * w
    qn = min(KW, TT - q0)   # query stream width (masked beyond 125)
    kn = min(KW, TT - q0)   # key window width
    return q0, qn, kn


def _mask_consts():
    """lmask [RANK,128] (lhsT), rmask [RANK,128*NW] (rhs):
    sum_r lmask[r,m]*rmask[r, 128w+j] = -C + C*[m//5 == j//5] for real in-window
    query cols j<125 (within bounds), -C for pad/overhang cols."""
    lm = np.zeros((RANK, KW), np.float32)
    lm[0, :] = 1.0
    segk = np.arange(KW) // 5          # 0..25 (seg 25 has no indicator row)
    for i in range(25):
        lm[1 + i, :] = (segk == i)
    rm = np.zeros((RANK, 128 * NW), np.float32)
    for w in range(NW):
        q0, qn, _ = _win_geom(w)
        nreal = min(WQ, TT - q0)       # real query cols in this window
        col = 128 * w
        rm[0, col:col + 128] = -CBF
        for j in range(nreal):
            rm[1 + (j // 5), col + j] = CBF
    return lm.astype(bf16), rm.astype(bf16)


def _fast_ok(ins):
    """Fast path: ln_out gain/bias scalar (and gain>0) for layers 0..L-2."""
    f32 = np.float32
    for l in range(L - 1):
        g = f32(ins["ln_out_g"][l])
        b = f32(ins["ln_out_b"][l])
        if not (np.all(g == g[0]) and g[0] > 0 and np.all(b == b[0])):
            return False
    return True


def _host_prep(ins, fast):
    """Fold LN affines/scales into weights, transpose, cast. Shared input map."""
    f32 = np.float32
    m = {}
    scale = np.float32(DH) ** -0.5
    for l in range(L):
        g_i, b_i = f32(ins["ln_in_g"][l]), f32(ins["ln_in_b"][l])
        g_f, b_f = f32(ins["ff_ln_g"][l]), f32(ins["ff_ln_b"][l])
        Wq = f32(ins["Wq"][l]);  bq = f32(ins["bq"][l])
        Wk = f32(ins["Wkv"][l][:D]);  bk = f32(ins["bkv"][l][:D])
        Wv = f32(ins["Wkv"][l][D:]);  bv = f32(ins["bkv"][l][D:])
        Wo = f32(ins["Wo"][l]);  bo = f32(ins["bo"][l])
        W1 = f32(ins["W1"][l]);  b1 = f32(ins["b1"][l])
        W2 = f32(ins["W2"][l]);  b2 = f32(ins["b2"][l])
        gp = f32(1.0)
        if fast and l > 0:
            gp = f32(ins["ln_out_g"][l - 1][0])   # scalar, >0 (checked)
        Wq_ = scale * (Wq * g_i[None, :]); bq_ = scale * (bq + Wq @ b_i)
        Wk_ = Wk * g_i[None, :];           bk_ = bk + Wk @ b_i
        Wv_ = Wv * g_i[None, :];           bv_ = bv + Wv @ b_i
        Wo_ = Wo / gp;                     bo_ = (bo + Wo @ bv_) / gp
        W1_ = W1 * g_f[None, :];           b1_ = b1 + W1 @ b_f
        W2_ = W2 / gp;                     b2_ = b2 / gp
        m[f"wq{l}"] = Wq_.T.copy().astype(bf16)   # [din, dout]
        m[f"wk{l}"] = Wk_.T.copy().astype(bf16)
        m[f"wv{l}"] = Wv_.T.copy().astype(bf16)
        m[f"wo{l}"] = Wo_.T.copy().astype(bf16)
        m[f"w1{l}"] = W1_.T.copy().astype(bf16)   # [512, 128]
        m[f"w2{l}"] = W2_.T.copy().astype(bf16)   # [128, 512]
        m[f"bq{l}"] = bq_.reshape(DT, 128).T.copy()       # [128, DT] f32
        m[f"bk{l}"] = bk_.reshape(DT, 128).T.copy()
        m[f"bo{l}"] = bo_.reshape(DT, 128).T.copy()
        m[f"b1{l}"] = b1_.reshape(1, FFN).T.copy()        # [128, 1]
        m[f"b2{l}"] = b2_.reshape(DT, 128).T.copy()
        m[f"w1r{l}"] = (-W1_.sum(axis=1)).reshape(1, FFN).T.copy()  # [128,1] -rowsum
        if not fast:
            m[f"go{l}"] = f32(ins["ln_out_g"][l]).reshape(DT, 128).T.copy()
            m[f"bo2{l}"] = f32(ins["ln_out_b"][l]).reshape(DT, 128).T.copy()
    lm, rm = _mask_consts()
    m["lmask"] = lm
    m["rmask"] = rm
    m["ones_c"] = np.full((128, 128), 1.0 / D, bf16)  # stats lhsT (bcast mean)
    m["allones"] = np.ones((128, 128), bf16)          # softmax denominator lhsT
    m["ident"] = np.eye(128, dtype=bf16)              # residual-add lhsT
    return m


def _dram_inputs(nc, fast):
    a = {}
    def inp(name, shape, dtype):
        a[name] = nc.dram_tensor(name, list(shape), dtype, kind="ExternalInput").ap()
    inp("xT", (D, TT), dt.bfloat16)
    for l in range(L):
        inp(f"wq{l}", (D, D), dt.bfloat16); inp(f"wk{l}", (D, D), dt.bfloat16)
        inp(f"wv{l}", (D, D), dt.bfloat16); inp(f"wo{l}", (D, D), dt.bfloat16)
        inp(f"w1{l}", (D, FFN), dt.bfloat16); inp(f"w2{l}", (FFN, D), dt.bfloat16)
        inp(f"bq{l}", (128, DT), dt.float32); inp(f"bk{l}", (128, DT), dt.float32)
        inp(f"bo{l}", (128, DT), dt.float32)
        inp(f"b1{l}", (128, 1), dt.float32); inp(f"b2{l}", (128, DT), dt.float32)
        inp(f"w1r{l}", (128, 1), dt.float32)
        if not fast:
            inp(f"go{l}", (128, DT), dt.float32)
            inp(f"bo2{l}", (128, DT), dt.float32)
    inp("lmask", (RANK, KW), dt.bfloat16)
    inp("rmask", (RANK, 128 * NW), dt.bfloat16)
    inp("ones_c", (128, 128), dt.bfloat16)
    inp("allones", (128, 128), dt.bfloat16)
    inp("ident", (128, 128), dt.bfloat16)
    out = nc.dram_tensor("out", [128, DT], dt.float32, kind="ExternalOutput").ap()
    return a, out


def _trace(nc, fast):
    a, out_dram = _dram_inputs(nc, fast)
    with tile.TileContext(nc) as tc:
        import contextlib
        ctx = contextlib.ExitStack()
        with ctx:
            consts = ctx.enter_context(tc.tile_pool(name="consts", bufs=1))
            wpool = ctx.enter_context(tc.tile_pool(name="w", bufs=2))
            acts = ctx.enter_context(tc.tile_pool(name="acts", bufs=1))
            small = ctx.enter_context(tc.tile_pool(name="small", bufs=2))
            psum = ctx.enter_context(tc.tile_pool(name="psum", bufs=1, space="PSUM"))

            # ---- constants ----
            smalls = {}
            for name, shape, dd in [
                ("lmask", [RANK, KW], dt.bfloat16),
                ("rmask", [RANK, 128 * NW], dt.bfloat16),
                ("ones_c", [128, 128], dt.bfloat16),
                ("allones", [128, 128], dt.bfloat16),
                ("ident", [128, 128], dt.bfloat16),
            ]:
                t = consts.tile(shape, dd, tag=name, name=name)
                nc.sync.dma_start(t[:], a[name])
                smalls[name] = t
            eps_tile = consts.tile([128, 1], dt.float32)
            nc.vector.memset(eps_tile[:], 1e-5)
            ones_c, allones = smalls["ones_c"], smalls["allones"]
            lmask, rmask = smalls["lmask"], smalls["rmask"]
            ident = smalls["ident"]

            def emit_sq(src, sq, c0, cn):
                """sq = src*src for one chunk, split across DVE/GPSIMD."""
                for d in range(DT):
                    eng = nc.gpsimd if d >= 2 else nc.vector
                    eng.tensor_tensor(sq[:, d, c0:c0 + cn], src[:, d, c0:c0 + cn],
                                      src[:, d, c0:c0 + cn], ALU.mult)

            def ln_stats_chunk(src, sq, c0, cn, need_A_bf):
                """Per-chunk LN stats: returns (pmu, mu_sb, A_f32slot, A_bf).
                pmu stays valid until its ring slot is reused."""
                pmu = psum.tile([128, 512], dt.float32, tag="pp", bufs=4, name="pmu")
                pe2 = psum.tile([128, 512], dt.float32, tag="pp", bufs=4, name="pe2")
                for d in range(DT):
                    nc.tensor.matmul(pmu[:, :cn], ones_c[:], src[:, d, c0:c0 + cn],
                                     start=(d == 0), stop=(d == DT - 1))
                for d in range(DT):
                    nc.tensor.matmul(pe2[:, :cn], ones_c[:], sq[:, d, c0:c0 + cn],
                                     start=(d == 0), stop=(d == DT - 1))
                mu_sb = small.tile([128, 512], dt.bfloat16, tag="musb", name="musb")
                nc.vector.tensor_copy(mu_sb[:, :cn], pmu[:, :cn])
                sqmu = small.tile([128, 512], dt.float32, tag="sqmu", name="sqmu")
                nc.vector.tensor_tensor(sqmu[:, :cn], mu_sb[:, :cn], mu_sb[:, :cn],
                                        ALU.mult)
                var = small.tile([128, 512], dt.float32, tag="var", name="var")
                nc.vector.scalar_tensor_tensor(var[:, :cn], sqmu[:, :cn], -1.0,
                                               pe2[:, :cn], ALU.mult, ALU.add)
                sd = small.tile([128, 512], dt.float32, tag="sd", name="sd")
                nc.scalar.activation(sd[:, :cn], var[:, :cn], AF.Sqrt,
                                     bias=eps_tile[:], scale=1.0)
                A = small.tile([128, 512], dt.float32, tag="A", bufs=3, name="A")
                nc.vector.reciprocal_approx_fast(A[:, :cn], sd[:, :cn])
                A_bf = None
                if need_A_bf:
                    A_bf = small.tile([128, 512], dt.bfloat16, tag="Abf", bufs=3,
                                      name="Abf")
                    nc.vector.tensor_copy(A_bf[:, :cn], A[:, :cn])
                return pmu, mu_sb, A, A_bf

            def ln_apply(src, dst):
                """dst = (src - mean) * rstd (per token)."""
                sq = acts.tile([128, DT, TT], dt.bfloat16, tag="sq", name="sq")
                for (c0, cn) in CHUNKS:
                    emit_sq(src, sq, c0, cn)
                    pmu, mu_sb, A, A_bf = ln_stats_chunk(src, sq, c0, cn, True)
                    for d in range(DT):
                        eng = nc.gpsimd if d >= 2 else nc.vector
                        t2 = small.tile([128, 512], dt.bfloat16, tag="t2", bufs=4,
                                        name="t2")
                        nc.vector.scalar_tensor_tensor(
                            t2[:, :cn], mu_sb[:, :cn], -1.0, src[:, d, c0:c0 + cn],
                            ALU.mult, ALU.add)
                        eng.tensor_tensor(dst[:, d, c0:c0 + cn], t2[:, :cn],
                                          A_bf[:, :cn], ALU.mult)

            # ---- initial residual: raw x (interleaved, transposed), chunked ----
            res = acts.tile([128, DT, TT], dt.bfloat16, tag="res", bufs=2, name="res")
            xTr = a["xT"].rearrange("(dtile p) t -> p dtile t", p=128)
            for (c0, cn) in CHUNKS:
                nc.sync.dma_start(res[:, :, c0:c0 + cn], xTr[:, :, c0:c0 + cn])

            for l in range(L):
                # ---- layer weights ----
                w = {}
                for nm, shape in [("wq", [128, DT, D]), ("wk", [128, DT, D]),
                                  ("wv", [128, DT, D]), ("wo", [128, DT, D]),
                                  ("w1", [128, DT, FFN])]:
                    t = wpool.tile(shape, dt.bfloat16, tag=nm, name=nm)
                    nc.sync.dma_start(t[:], a[f"{nm}{l}"].rearrange(
                        "(dtile p) o -> p dtile o", p=128))
                    w[nm] = t
                w["w2"] = wpool.tile([128, D], dt.bfloat16, tag="w2", name="w2")
                nc.sync.dma_start(w["w2"][:], a[f"w2{l}"])
                bias = {}
                bnames = ["bq", "bk", "bo", "b1", "b2", "w1r"] + (
                    [] if fast else ["go", "bo2"])
                for nm in bnames:
                    t = wpool.tile([128, DT] if nm not in ("b1", "w1r") else [128, 1],
                                   dt.float32, tag=nm, name=nm)
                    nc.sync.dma_start(t[:], a[f"{nm}{l}"])
                    bias[nm] = t

                # ---- ln_in (explicit for layer 0 / general path) ----
                if l == 0 or not fast:
                    zq = acts.tile([128, DT, TT], dt.bfloat16, tag="zq", name="zq")
                    ln_apply(res, zq)
                else:
                    zq = res

                # ---- Q, K projections (weights stationary, transposed out) ----
                qk = {}
                for nm, bnm, tg in [("wq", "bq", "qt"), ("wk", "bk", "kt")]:
                    dst = acts.tile([128, DT, TT], dt.bfloat16, tag=tg, name=tg)
                    for o in range(DT):
                        for ci, (c0, cn) in enumerate(CHUNKS):
                            p = psum.tile([128, 512], dt.float32, tag="pp", bufs=4,
                                          name="pqk")
                            for d in range(DT):
                                nc.tensor.matmul(
                                    p[:, :cn],
                                    w[nm][:, d, 128 * o:128 * o + 128],
                                    zq[:, d, c0:c0 + cn],
                                    start=(d == 0), stop=(d == DT - 1))
                            if (o + ci) % 2 == 0:
                                nc.scalar.activation(dst[:, o, c0:c0 + cn], p[:, :cn],
                                                     AF.Identity,
                                                     bias=bias[bnm][:, o:o + 1],
                                                     scale=1.0)
                            else:
                                nc.vector.tensor_scalar(dst[:, o, c0:c0 + cn],
                                                        p[:, :cn],
                                                        bias[bnm][:, o:o + 1], None,
                                                        ALU.add)
                    qk[nm] = dst
                q_t, k_t = qk["wq"], qk["wk"]

                # ---- V in overlapping 128-token key windows (no bias: folded) ----
                v_win = acts.tile([128, NW, D], dt.bfloat16, tag="vw", name="vw")
                for wi in range(NW):
                    kw0, _, kn = _win_geom(wi)
                    p = psum.tile([128, 512], dt.float32, tag="pp", bufs=4, name="pv")
                    for d in range(DT):
                        nc.tensor.matmul(p[0:kn, :], zq[:, d, kw0:kw0 + kn],
                                         w["wv"][:, d, :],
                                         start=(d == 0), stop=(d == DT - 1))
                    nc.scalar.activation(v_win[0:kn, wi, :], p[0:kn, :], AF.Identity)

                # ---- attention: per head, per 4-window group ----
                attn = acts.tile([128, DT, TT], dt.bfloat16, tag="at", name="at")
                for h in range(H):
                    for g, wlist in enumerate(GROUPS):
                        ng = 128 * len(wlist)
                        ps = psum.tile([128, 512], dt.float32, tag="ps", bufs=2,
                                       name="ps")
                        nc.tensor.matmul(ps[:, :ng], lmask[:],
                                         rmask[:, 512 * g:512 * g + ng],
                                         start=True, stop=False)
                        for wi in wlist:
                            q0, qn, kn = _win_geom(wi)
                            ow = 128 * (wi - wlist[0])
                            nc.tensor.matmul(ps[0:kn, ow:ow + qn],
                                             k_t[:, h, q0:q0 + kn],
                                             q_t[:, h, q0:q0 + qn],
                                             start=False, stop=True)
                        pa = small.tile([128, 512], dt.bfloat16, tag="pa", bufs=3,
                                        name="pa")
                        nc.scalar.activation(pa[:, :ng], ps[:, :ng], AF.Exp)
                        pd = psum.tile([128, 512], dt.float32, tag="pd", bufs=1,
                                       name="pd")
                        nc.tensor.matmul(pd[:, :ng], allones[:], pa[:, :ng],
                                         start=True, stop=True)
                        rec = small.tile([128, 512], dt.float32, tag="rec", name="rec")
                        nc.vector.reciprocal_approx_fast(rec[:, :ng], pd[:, :ng])
                        pav = psum.tile([128, 512], dt.float32, tag="pav", bufs=1,
                                        name="pav")
                        for wi in wlist:
                            q0, qn, kn = _win_geom(wi)
                            ow = 128 * (wi - wlist[0])
                            nc.tensor.matmul(pav[:, ow:ow + qn],
                                             v_win[0:kn, wi, 128 * h:128 * h + 128],
                                             pa[0:kn, ow:ow + qn],
                                             start=True, stop=True)
                        # normalize + compact (drop per-window pad/overhang cols)
                        full = [wi for wi in wlist if WQ * wi + WQ <= TT]
                        nf = len(full)
                        qg0 = WQ * wlist[0]
                        if nf:
                            pav_v = pav[:, :].rearrange("p (w j) -> p w j", j=128)
                            rec_v = rec[:, :].rearrange("p (w j) -> p w j", j=128)
                            nc.vector.tensor_tensor(
                                attn[:, h, qg0:qg0 + WQ * nf],
                                pav_v[:, 0:nf, 0:WQ], rec_v[:, 0:nf, 0:WQ], ALU.mult)
                        for wi in wlist[nf:]:          # partial tail window
                            q0, qn, _ = _win_geom(wi)
                            ow = 128 * (wi - wlist[0])
                            nc.vector.tensor_tensor(
                                attn[:, h, q0:TT],
                                pav[:, ow:ow + (TT - q0)],
                                rec[:, ow:ow + (TT - q0)], ALU.mult)

                # ---- Wo projection + bias + residual (fused drain) ----
                rc = acts.tile([128, DT, TT], dt.bfloat16, tag="rc", name="rc")
                for o in range(DT):
                    for (c0, cn) in CHUNKS:
                        p = psum.tile([128, 512], dt.float32, tag="pp", bufs=4,
                                      name="pwo")
                        for d in range(DT):
                            nc.tensor.matmul(p[:, :cn],
                                             w["wo"][:, d, 128 * o:128 * o + 128],
                                             attn[:, d, c0:c0 + cn],
                                             start=(d == 0), stop=False)
                        nc.tensor.matmul(p[:, :cn], ident[:],
                                         res[:, o, c0:c0 + cn],
                                         start=False, stop=True)
                        nc.scalar.activation(rc[:, o, c0:c0 + cn], p[:, :cn],
                                             AF.Identity,
                                             bias=bias["bo"][:, o:o + 1], scale=1.0)

                # ---- FFN with ff_ln folded into the drains ----
                # h1 = relu((W1@rc - w1_rowsum*mu)*A + b1); W1 needs no LN stats.
                sqf = acts.tile([128, DT, TT], dt.bfloat16, tag="sq", name="sqf")
                h1 = acts.tile([128, TT], dt.bfloat16, tag="h1", name="h1")
                for (c0, cn) in CHUNKS:
                    p1 = psum.tile([128, 512], dt.float32, tag="pp", bufs=4,
                                   name="p1")
                    for d in range(DT):
                        nc.tensor.matmul(p1[:, :cn], w["w1"][:, d, :],
                                         rc[:, d, c0:c0 + cn],
                                         start=(d == 0), stop=(d == DT - 1))
                    emit_sq(rc, sqf, c0, cn)
                    pmu, mu_sb, A, _ = ln_stats_chunk(rc, sqf, c0, cn, False)
                    muA = small.tile([128, 512], dt.bfloat16, tag="muA", name="muA")
                    nc.vector.tensor_tensor(muA[:, :cn], pmu[:, :cn], A[:, :cn],
                                            ALU.mult)
                    t1 = small.tile([128, 512], dt.bfloat16, tag="t1", name="t1")
                    nc.vector.tensor_tensor(t1[:, :cn], p1[:, :cn], A[:, :cn],
                                            ALU.mult)
                    hp = small.tile([128, 512], dt.bfloat16, tag="hp", name="hp")
                    nc.vector.scalar_tensor_tensor(hp[:, :cn], muA[:, :cn],
                                                   bias["w1r"][:], t1[:, :cn],
                                                   ALU.mult, ALU.add)
                    nc.vector.tensor_scalar(h1[:, c0:c0 + cn], hp[:, :cn],
                                            bias["b1"][:], 0.0, ALU.add, ALU.max)
                y = acts.tile([128, DT, TT], dt.bfloat16, tag="y", name="y")
                for o in range(DT):
                    for (c0, cn) in CHUNKS:
                        p = psum.tile([128, 512], dt.float32, tag="pp", bufs=4,
                                      name="p2")
                        nc.tensor.matmul(p[:, :cn], w["w2"][:, 128 * o:128 * o + 128],
                                         h1[:, c0:c0 + cn], start=True, stop=False)
                        nc.tensor.matmul(p[:, :cn], ident[:],
                                         rc[:, o, c0:c0 + cn],
                                         start=False, stop=True)
                        nc.scalar.activation(y[:, o, c0:c0 + cn], p[:, :cn],
                                             AF.Identity,
                                             bias=bias["b2"][:, o:o + 1], scale=1.0)

                # ---- ln_out -> next residual (normalized s-stream) ----
                s_next = acts.tile([128, DT, TT], dt.bfloat16, tag="res", bufs=2,
                                   name="snext")
                ln_apply(y, s_next)
                if not fast:
                    for d in range(DT):
                        nc.vector.tensor_scalar(s_next[:, d], s_next[:, d],
                                                bias["go"][:, d:d + 1],
                                                bias["bo2"][:, d:d + 1],
                                                ALU.mult, ALU.add)
                res = s_next

            # ---- mean-pool utterance tokens (pos 1..4 of each 5-block) ----
            out_sb = small.tile([128, DT], dt.float32, tag="outsb", name="outsb")
            for d in range(DT):
                view = res[:, d, :].rearrange("p (s j) -> p s j", j=5)[:, :, 1:5]
                nc.vector.tensor_reduce(out_sb[:, d:d + 1], view,
                                        axis=mybir.AxisListType.XY, op=ALU.add)
            nc.vector.tensor_scalar_mul(out_sb[:], out_sb[:], 1.0 / U)
            nc.sync.dma_start(out_dram, out_sb[:])
    nc.compile()
    return nc


def _build(fast):
    nc = bacc.Bacc("TRN2", target_bir_lowering=False, debug=False, num_devices=NCORES)
    return _trace(nc, fast)


def kernel(**inputs):
    global _COMPILED, _FAST
    ins = {k: np.asarray(v) for k, v in inputs.items()}
    fast = _fast_ok(ins)
    shared = _host_prep(ins, fast)
    idx = _tok_index()
    x = ins["x"].astype(np.float32)          # [B, T, D]
    xp = x[:, idx, :]                        # [B, TT, D]
    xT = np.ascontiguousarray(xp.transpose(0, 2, 1)).astype(bf16)  # [B, D, TT]
    if _COMPILED is None or _FAST != fast:
        _COMPILED = _build(fast)
        _FAST = fast
    nc = _COMPILED
    in_maps = []
    for b in range(NCORES):
        m = dict(shared)
        m["xT"] = xT[b]
        in_maps.append(m)
    res = bass_utils.run_bass_kernel_spmd(nc, in_maps, core_ids=list(range(NCORES)))
    outs = []
    for b in range(NCORES):
        o = res.results[b]["out"]            # [128, DT]
        outs.append(o.T.reshape(D))          # d = dtile*128 + p
    out = np.stack(outs).astype(np.float32)
    if fast:
        g3 = np.float32(ins["ln_out_g"][L - 1])
        b3 = np.float32(ins["ln_out_b"][L - 1])
        out = out * g3[None, :] + b3[None, :]
    return out
